# revision 2
# baseline (speedup 1.0000x reference)
"""BloomAttention (B=1, S=2048, HID=4096, NH=32) on 8 Trainium2 NeuronCores.

Strategy (tensor-parallel over heads, as the module does):
  - Each core owns 4 heads. w_qkv/b_qkv column-sharded (per-head q/k/v rows),
    INV_NORM folded into the q slice on host; weights shipped transposed+bf16,
    activations shipped bf16 (compute dtype).
  - On-device: hiddenT tiles via xbar DMA-transpose; QKV matmul produces
    qT/kT [d, s] per head directly, V staged to DRAM and transpose-loaded
    back as natural [s, d] for the PV matmul.
  - Attention in transposed-scores layout: scoresT[sk, sq] = kT.T @ qT.
    ALiBi bias + per-query shift + causal mask are all applied in ONE vector
    op per tile: ps += slope_h * D[a,b] where D = (sk - sq) on causal-valid
    entries and -4e9 on masked ones. D depends only on the 128-aligned tile
    offset (19 distinct tiles, SBUF-resident). The shift (-slope*sq) is
    exact: softmax is shift-invariant per query, and the diagonal term
    bounds exp() so no max-reduce is needed. exp on ACT; P@V and the
    softmax denominator are matmuls over the sk partitions (ones column),
    software-pipelined behind the score matmuls; normalization uses a
    ones-row broadcast matmul + reciprocal.
  - AllToAll swaps head-shards for sequence-shards of the context, then each
    core computes its 256 output rows against the full (transposed, bf16)
    w_dense. Host just concatenates the 8 row-shards.

Note: assumes the alibi input is the standard Bloom form alibi[h, j] =
slope_h * j (slope read from alibi[:, 1]); the reference's setup_inputs
builds exactly that.
"""

import math
import os
import sys
import types
from contextlib import ExitStack

import numpy as np
import ml_dtypes

B, S, HID, NH, HD = 1, 2048, 4096, 32, 128
NCORES = 8
NH_LOC = NH // NCORES            # 4 heads per core
FQKV = NH_LOC * 3 * HD           # 1536 qkv features per core
SROW = S // NCORES               # 256 output rows per core
INV_NORM = 1.0 / math.sqrt(HD)
KT = HID // HD                   # 32 k tiles
KC = 12                          # k tiles cached in SBUF (rest streamed)
KS = KT - KC                     # streamed k tiles
NR = 19                          # distinct (sk-sq)/128 tile offsets: -15..3

_CACHE = {}


def _ensure_axon_hooks():
    try:
        import antenv  # noqa: F401

        extra = "/opt/trn_rl_repo/antenv"
        if os.path.isdir(extra) and extra not in antenv.__path__:
            antenv.__path__.append(extra)
        import antenv.axon_hooks  # noqa: F401
    except Exception:
        hook = None
        try:
            from trn_agent_boot.trn_boot import _ntff_profile_via_ctypes

            hook = _ntff_profile_via_ctypes("/opt/axon/libaxon_pjrt.so")
        except Exception:
            hook = None
        m = types.ModuleType("antenv.axon_hooks")
        m._hook = hook
        m.get_axon_ntff_profile_hook = lambda: m._hook
        m.set_axon_ntff_profile_hook = lambda h: setattr(m, "_hook", h)
        sys.modules["antenv.axon_hooks"] = m


def _kt_order():
    cached = list(range(KC))
    streamed = list(range(KC, KT))
    order = []
    for i in range(max(len(cached), len(streamed))):
        if i < len(cached):
            order.append(cached[i])
        if i < len(streamed):
            order.append(streamed[i])
    return order


def _build_nc():
    import concourse.bass as bass  # noqa: F401
    import concourse.mybir as mybir
    from concourse import bacc, tile

    BF = mybir.dt.bfloat16
    F32 = mybir.dt.float32
    Alu = mybir.AluOpType
    Act = mybir.ActivationFunctionType

    nc = bacc.Bacc(None, target_bir_lowering=False, num_devices=NCORES)
    with tile.TileContext(nc) as tc, ExitStack() as ctx:
        dram = ctx.enter_context(tc.tile_pool(name="dram", bufs=1, space="DRAM"))

        def din(name, shape, dt):
            return dram.tile(shape, dt, kind="ExternalInput", name=name,
                             uniquify=False)

        hidden = din("hidden", [S, HID], BF)
        wqcd = din("wqc", [HD, KC, FQKV], BF)
        wstrd = din("wstr", [2, HD, KS, 768], BF)
        bqkv = din("bqkv", [HD, NH_LOC * 3], F32)
        dmatd = din("dmat", [HD, NR * 512], F32)
        slopesd = din("slopes", [HD, NH_LOC], F32)
        wdr = din("wdr", [8, HD, KT, 512], BF)
        bdense = din("bdense", [1, HID], F32)
        out = dram.tile([SROW, HID], F32, kind="ExternalOutput", name="out",
                        uniquify=False)
        a2a_in = [dram.tile([NCORES, 2, HD, SROW], BF, name=f"a2a_in{p}")
                  for p in range(2)]
        a2a_out = [dram.tile([NCORES, 2, HD, SROW], BF, name=f"a2a_out{p}")
                   for p in range(2)]
        vdram = dram.tile([NH_LOC, HD, S], BF, name="vdram")

        # ---------- persistent SBUF ----------
        const = ctx.enter_context(tc.tile_pool(name="const", bufs=1))
        sb_bqkv = const.tile([HD, NH_LOC * 3], F32)
        nc.sync.dma_start(out=sb_bqkv[:], in_=bqkv[:])
        sb_slopes = const.tile([HD, NH_LOC], F32)
        nc.sync.dma_start(out=sb_slopes[:], in_=slopesd[:])
        ones_col = const.tile([HD, 1], BF)
        nc.vector.memset(ones_col[:], 1.0)
        ones_row = const.tile([1, HD], F32)
        nc.vector.memset(ones_row[:], 1.0)

        persist = ctx.enter_context(tc.tile_pool(name="persist", bufs=1))
        qT = [persist.tile([HD, S], BF, name=f"qT{h}") for h in range(NH_LOC)]
        kTt = [persist.tile([HD, S], BF, name=f"kT{h}") for h in range(NH_LOC)]
        vnat = [persist.tile([HD, S], BF, name=f"vn{h}")
                for h in range(NH_LOC)]

        # ---------- phase 1: QKV ----------
        FG = [list(range(0, 6)), list(range(6, 12))]
        KORD = _kt_order()
        with (
            tc.tile_pool(name="wqc", bufs=1) as wqc_pool,
            tc.tile_pool(name="wstream", bufs=2) as ws_pool,
            tc.tile_pool(name="hT", bufs=2) as hT_pool,
            tc.tile_pool(name="vstg", bufs=3) as vstg_pool,
            tc.tile_pool(name="qkv_ps", bufs=1, space="PSUM") as qkv_ps,
        ):
            wq_c = wqc_pool.tile([HD, KC, FQKV], BF)
            nc.sync.dma_start(out=wq_c[:], in_=wqcd[:])

            for sq in range(4):  # s-quarters of 512
                s0 = sq * 512
                hT_q = hT_pool.tile([HD, KT, 512], BF, name="hT_q")
                for kt in KORD:
                    nc.scalar.dma_start(
                        out=hT_q[:, kt, :],
                        in_=hidden[s0:s0 + 512, kt * HD:(kt + 1) * HD],
                        transpose=True)
                for fg in FG:
                    nf = len(fg)
                    f0 = fg[0] * HD
                    psl = [qkv_ps.tile([HD, 512], F32, name=f"qkvps{i}",
                                       bufs=1) for i in range(nf)]
                    # two big prefetch DMAs for the streamed half of K
                    fgi = fg[0] // 6
                    half_n = KS // 2
                    wsts = []
                    for half in range(2):
                        k0 = half * half_n
                        wst = ws_pool.tile([HD, half_n, 6 * HD], BF,
                                           name="ws")
                        nc.sync.dma_start(
                            out=wst[:],
                            in_=wstrd[fgi, :, k0:k0 + half_n, :])
                        wsts.append(wst)
                    for ki, kt in enumerate(KORD):
                        if kt < KC:
                            wsl = wq_c[:, kt, f0:f0 + nf * HD]
                        else:
                            wsl = wsts[(kt - KC) // half_n][
                                :, (kt - KC) % half_n, :]
                        for i in range(nf):
                            nc.tensor.matmul(
                                psl[i][:],
                                wsl[:, i * HD:(i + 1) * HD],
                                hT_q[:, kt, :],
                                start=(ki == 0), stop=(ki == KT - 1))
                    for i, ft in enumerate(fg):
                        h, j = divmod(ft, 3)
                        if j < 2:
                            dest = (qT, kTt)[j][h][:, s0:s0 + 512]
                            nc.scalar.activation(
                                dest, psl[i][:], Act.Identity,
                                bias=sb_bqkv[:, ft:ft + 1])
                        else:
                            vs = vstg_pool.tile([HD, 512], BF, name="vs")
                            nc.scalar.activation(
                                vs[:], psl[i][:], Act.Identity,
                                bias=sb_bqkv[:, ft:ft + 1])
                            nc.sync.dma_start(
                                out=vdram[h, :, s0:s0 + 512], in_=vs[:])
                            for t4 in range(4):
                                sk0 = s0 + t4 * HD
                                nc.scalar.dma_start(
                                    out=vnat[h][:, sk0:sk0 + HD],
                                    in_=vdram[h, :, sk0:sk0 + HD],
                                    transpose=True)

        # ---------- phase 2: attention ----------
        with (
            tc.tile_pool(name="attn_sb", bufs=1) as attn_sb,
            tc.tile_pool(name="expp", bufs=4) as expp,
            tc.tile_pool(name="bcp", bufs=2) as bcp,
            tc.tile_pool(name="attn_ps", bufs=1, space="PSUM") as attn_ps,
            tc.tile_pool(name="sc_ps", bufs=4, space="PSUM") as sc_ps,
        ):
            dmat = attn_sb.tile([HD, NR * 512], F32)
            nc.sync.dma_start(out=dmat[:], in_=dmatd[:])
            ctxT = [attn_sb.tile([HD, S], BF, name=f"cx{h}")
                    for h in range(NH_LOC)]

            for h in range(NH_LOC):
                slope = sb_slopes[:, h:h + 1]
                for sqb in range(4):
                    q0 = sqb * 512
                    nsk = 4 * (sqb + 1)
                    ps_ctx = attn_ps.tile([HD, 512], F32, name="ps_ctx", bufs=2)
                    ps_sum = attn_ps.tile([1, 512], F32, name="ps_sum", bufs=1)
                    exs = {}

                    def flush(skt, first, last):
                        ex = exs.pop(skt)
                        nc.tensor.matmul(
                            ps_ctx[:], vnat[h][:, skt * HD:(skt + 1) * HD],
                            ex[:], start=first, stop=last)
                        nc.tensor.matmul(
                            ps_sum[:], ones_col[:], ex[:],
                            start=first, stop=last)

                    for skt in range(nsk):
                        ri = skt - 4 * sqb + 15  # (sk0-q0)/128 + 15
                        ps = sc_ps.tile([HD, 512], F32, name="ps_sc")
                        nc.tensor.matmul(
                            ps[:], kTt[h][:, skt * HD:(skt + 1) * HD],
                            qT[h][:, q0:q0 + 512], start=True, stop=True)
                        nc.vector.scalar_tensor_tensor(
                            ps[:], dmat[:, ri * 512:(ri + 1) * 512], slope,
                            ps[:], Alu.mult, Alu.add)
                        ex = expp.tile([HD, 512], BF, name="ex")
                        nc.scalar.activation(ex[:], ps[:], Act.Exp)
                        exs[skt] = ex
                        if skt >= 2:
                            flush(skt - 2, skt - 2 == 0, False)
                    for skt in (nsk - 2, nsk - 1):
                        flush(skt, skt == 0, skt == nsk - 1)

                    ps_bc = attn_ps.tile([HD, 512], F32, name="ps_bc", bufs=1)
                    sum_sb = bcp.tile([1, 512], F32, name="sum_sb")
                    nc.scalar.copy(sum_sb[:], ps_sum[:])
                    nc.tensor.matmul(ps_bc[:], ones_row[:], sum_sb[:],
                                     start=True, stop=True)
                    rec_bc = bcp.tile([HD, 512], F32, name="rec_bc")
                    nc.vector.reciprocal(rec_bc[:], ps_bc[:])
                    nc.vector.tensor_tensor(
                        ctxT[h][:, q0:q0 + 512], ps_ctx[:], rec_bc[:],
                        Alu.mult)
                    for j in (2 * sqb, 2 * sqb + 1):
                        nc.sync.dma_start(
                            out=a2a_in[h // 2][j, h % 2],
                            in_=ctxT[h][:, j * SROW:(j + 1) * SROW])

            # ---------- phase 3: all-to-all ----------
            for p in range(2):
                nc.gpsimd.collective_compute(
                    "AllToAll", Alu.bypass,
                    replica_groups=[list(range(NCORES))],
                    ins=[a2a_in[p][:]], outs=[a2a_out[p][:]],
                )

        # ---------- phase 4: dense ----------
        with (
            tc.tile_pool(name="dns_sb", bufs=1) as dns_sb,
            tc.tile_pool(name="wd_pool", bufs=2) as wd_pool,
            tc.tile_pool(name="osb_pool", bufs=3) as osb_pool,
            tc.tile_pool(name="dns_ps", bufs=3, space="PSUM") as dns_ps,
        ):
            sb_bd = dns_sb.tile([1, HID], F32)
            nc.sync.dma_start(out=sb_bd[:], in_=bdense[:])
            crecv = dns_sb.tile([HD, KT, SROW], BF)
            for i in range(NCORES):
                for p in range(2):
                    nc.sync.dma_start(
                        out=crecv[:, i * NH_LOC + p * 2:
                                  i * NH_LOC + p * 2 + 2, :],
                        in_=a2a_out[p][i].rearrange("l p s -> p l s"))
            for ot in range(8):
                o0 = ot * 512
                wd = wd_pool.tile([HD, KT, 512], BF, name="wd")
                nc.sync.dma_start(out=wd[:], in_=wdr[ot])
                for st in range(2):
                    psd = dns_ps.tile([HD, 512], F32, name="psd")
                    for ft in range(KT):
                        nc.tensor.matmul(
                            psd[:], crecv[:, ft, st * HD:(st + 1) * HD],
                            wd[:, ft, :], start=(ft == 0), stop=False)
                    nc.tensor.matmul(
                        psd[:], ones_row[:], sb_bd[:, o0:o0 + 512],
                        start=False, stop=True)
                    osb = osb_pool.tile([HD, 512], F32, name="osb")
                    nc.scalar.copy(osb[:], psd[:])
                    nc.sync.dma_start(
                        out=out[st * HD:(st + 1) * HD, o0:o0 + 512],
                        in_=osb[:])
    nc.compile()
    return nc


def _prep_shards(hidden_states, alibi, w_qkv, b_qkv, w_dense, b_dense):
    bf16 = ml_dtypes.bfloat16
    hidden = np.ascontiguousarray(
        np.asarray(hidden_states, dtype=np.float32).reshape(S, HID)
    ).astype(bf16)
    al = np.asarray(alibi, dtype=np.float32).reshape(NH, S)
    w = np.asarray(w_qkv, dtype=np.float32)
    b = np.asarray(b_qkv, dtype=np.float32)
    wd = np.asarray(w_dense, dtype=np.float32)
    bd = np.asarray(b_dense, dtype=np.float32)

    # fold INV_NORM into the q projections
    scale = np.ones(3 * HID, np.float32)
    for h in range(NH):
        scale[h * 3 * HD:(h * 3 * HD) + HD] = INV_NORM
    wT = np.ascontiguousarray((w * scale[:, None]).T)      # [HID, 3*HID]
    bs = b * scale
    # dense weight, transposed then tiled [8 ot][32 ft][128 f][512 o]
    wdT = np.ascontiguousarray(wd.T).astype(bf16)          # [HID(f), HID(o)]
    wdr = np.ascontiguousarray(
        wdT.reshape(KT, HD, 8, 512).transpose(2, 1, 0, 3))
    bdr = np.ascontiguousarray(bd.reshape(1, HID))

    # D tiles: for r-offset index ri (0..18), D[a, b] = (ri-15)*128 + a - b
    # where causal-valid (<= 0), else -4e9
    a = np.arange(HD)[:, None]
    bq = np.arange(512)[None, :]
    dm = []
    for ri in range(NR):
        dv = ((ri - 15) * HD + a - bq).astype(np.float32)
        dm.append(np.where(dv <= 0, dv, np.float32(-4.0e9)))
    dmat = np.concatenate(dm, axis=1)                       # [128, 19*512]

    in_maps = []
    for c in range(NCORES):
        f0 = c * FQKV
        heads = list(range(c * NH_LOC, (c + 1) * NH_LOC))
        alc = al[heads]                                     # [4, S]
        slopes = np.repeat(alc[:, 1:2].T, HD, axis=0)       # [128, 4]
        wTc = wT[:, f0:f0 + FQKV].astype(bf16)              # [HID, 1536]
        # cached half: [128, KC, 1536] partition-contiguous
        wqc = np.ascontiguousarray(
            wTc[:KC * HD].reshape(KC, HD, FQKV).transpose(1, 0, 2))
        # streamed half, pre-split by fg column group: [2, 128, KS, 768]
        wstr = np.ascontiguousarray(
            wTc[KC * HD:].reshape(KS, HD, 2, 768).transpose(2, 1, 0, 3))
        in_maps.append({
            "hidden": hidden,
            "wqc": wqc,
            "wstr": wstr,
            "bqkv": np.ascontiguousarray(
                bs[f0:f0 + FQKV].reshape(NH_LOC * 3, HD).T),
            "dmat": dmat,
            "slopes": np.ascontiguousarray(slopes.astype(np.float32)),
            "wdr": wdr,
            "bdense": bdr,
        })
    return in_maps


def kernel(hidden_states, alibi, w_qkv, b_qkv, w_dense, b_dense):
    _ensure_axon_hooks()
    from concourse import bass_utils

    if "nc" not in _CACHE:
        _CACHE["nc"] = _build_nc()
    nc = _CACHE["nc"]
    in_maps = _prep_shards(hidden_states, alibi, w_qkv, b_qkv,
                           w_dense, b_dense)
    trace = bool(os.environ.get("BLOOM_TRACE"))
    res = bass_utils.run_bass_kernel_spmd(
        nc, in_maps, core_ids=list(range(NCORES)), trace=trace)
    kernel._last_results = res
    kernel._last_exec_ns = res.exec_time_ns
    outp = np.concatenate([res.results[c]["out"] for c in range(NCORES)],
                          axis=0)
    return outp.reshape(B, S, HID).astype(np.float32)



# revision 10
# speedup vs baseline: 1.2859x; 1.2859x over previous
"""BloomAttention (B=1, S=2048, HID=4096, NH=32) on 8 Trainium2 NeuronCores.

v2 strategy (tensor-parallel over heads):
  - Heads assigned per core by octile slots: core c owns global heads
    {c, c+8, c+16, c+24}. Slot k's ALiBi slope is at most 2^-(2k+2), so
    attention blocks farther than D_slot = 30/slope_min from the causal
    diagonal contribute < ~1e-7 relative mass and are skipped (structure is
    baked uniformly across cores; slope VALUES stay per-core runtime data).
  - hidden is transposed on HOST (hT [HID, S]) - zero on-device transposes.
  - Flash order: per 512-quarter, QKV matmuls then attention. q/k produced
    feature-major (weights stationary); V produced sequence-major directly
    (hT blocks stationary, w_v moving) so PV needs no transpose.
  - Scores in transposed layout [sk, sq]: alibi+causal+per-query shift via a
    single wide masked distance table T[a,c] = (a-c <= 0 ? a-c : -60000),
    applied by one DVE scalar_tensor_tensor per (column-striped) block; exp
    on ACT; denominator via ones[128,128] stationary matmul accumulating a
    broadcast row-sum in PSUM; 1/d computed as exp(-ln d) on ACT (one act
    table set serves exp+ln+identity); V-bias folded into b_dense on host.
  - AllToAll (2x 1MB) swaps head-shards for sequence-shards; first fires
    after local heads {c, c+8} finish. Dense: w_dense streamed in o-chunks,
    crecv stationary (LDW amortized over o), p0/p1 split on the first chunk
    to hide the second collective; bias added via hi/lo bf16 ones-matmuls.
"""

import math
import os
import sys
import types
from contextlib import ExitStack

import numpy as np
import ml_dtypes

B, S, HID, NH, HD = 1, 2048, 4096, 32, 128
NCORES = 8
NH_LOC = NH // NCORES            # 4 heads per core (slots)
SROW = S // NCORES               # 256 output rows per core
INV_NORM = 1.0 / math.sqrt(HD)
KT = HID // HD                   # 32 k tiles
TW = 2432                        # wide distance-table columns
NEG = -60000.0
DSLOT = [120, 480, 1920, 2048]   # per-slot causal stripe depth (30/slope_min)

_CACHE = {}


def _ensure_axon_hooks():
    try:
        import antenv  # noqa: F401

        extra = "/opt/trn_rl_repo/antenv"
        if os.path.isdir(extra) and extra not in antenv.__path__:
            antenv.__path__.append(extra)
        import antenv.axon_hooks  # noqa: F401
    except Exception:
        hook = None
        try:
            from trn_agent_boot.trn_boot import _ntff_profile_via_ctypes

            hook = _ntff_profile_via_ctypes("/opt/axon/libaxon_pjrt.so")
        except Exception:
            hook = None
        m = types.ModuleType("antenv.axon_hooks")
        m._hook = hook
        m.get_axon_ntff_profile_hook = lambda: m._hook
        m.set_axon_ntff_profile_hook = lambda h: setattr(m, "_hook", h)
        sys.modules["antenv.axon_hooks"] = m


def _surv(hl, q):
    """Surviving (skt, vs0, ve) column stripes for local head hl, quarter q.

    First surviving block is widened to the full 512 columns so its
    start=True matmul initialises every PSUM column of ps_ctx / ps_bc.
    """
    D = DSLOT[hl]
    sq0 = 512 * q
    out = []
    for skt in range(4 * q + 4):
        vs0 = max(0, 128 * skt - sq0)
        ve = min(512, 128 * skt + 128 + D - sq0)
        if ve <= vs0:
            continue
        out.append([skt, vs0, ve])
    out[0][1] = 0
    out[0][2] = 512
    return out


def _build_nc():
    import concourse.bass as bass  # noqa: F401
    import concourse.mybir as mybir
    from concourse import bacc, tile

    BF = mybir.dt.bfloat16
    F16 = mybir.dt.float16
    F32 = mybir.dt.float32
    Alu = mybir.AluOpType
    Act = mybir.ActivationFunctionType

    nc = bacc.Bacc(None, target_bir_lowering=False, num_devices=NCORES)
    with tile.TileContext(nc) as tc, ExitStack() as ctx:
        dram = ctx.enter_context(tc.tile_pool(name="dram", bufs=1, space="DRAM"))

        def din(name, shape, dt):
            return dram.tile(shape, dt, kind="ExternalInput", name=name,
                             uniquify=False)

        hTd = din("hT", [HD, KT, S], BF)
        wqkd = din("wqk", [HD, KT, 8 * HD], BF)
        wvd = din("wv", [HD, KT, 4 * HD], BF)
        bqkd = din("bqk", [HD, 8], F32)
        tmatd = din("tmat", [HD, TW], F16)
        slopesd = din("slopes", [HD, NH_LOC], F32)
        wdd = din("wd", [8, HD, KT, 512], BF)
        bdhd = din("bdh", [1, HID], BF)
        bdld = din("bdl", [1, HID], BF)
        out = dram.tile([SROW, HID], F32, kind="ExternalOutput", name="out",
                        uniquify=False)
        a2a_in = [dram.tile([NCORES, 2, HD, SROW], BF, name=f"a2a_in{p}")
                  for p in range(2)]
        a2a_out = [dram.tile([NCORES, 2, HD, SROW], BF, name=f"a2a_out{p}")
                   for p in range(2)]

        # ---------- persistent SBUF ----------
        const = ctx.enter_context(tc.tile_pool(name="const", bufs=1))
        sb_bqk = const.tile([HD, 8], F32)
        nc.sync.dma_start(out=sb_bqk[:], in_=bqkd[:])
        sb_slopes = const.tile([HD, NH_LOC], F32)
        nc.sync.dma_start(out=sb_slopes[:], in_=slopesd[:])
        tmat = const.tile([HD, TW], F16)
        nc.sync.dma_start(out=tmat[:], in_=tmatd[:])
        ones128 = const.tile([HD, HD], BF)
        nc.vector.memset(ones128[:], 1.0)
        ones1 = const.tile([1, HD], BF)
        nc.vector.memset(ones1[:], 1.0)
        sb_bdh = const.tile([1, HID], BF)
        nc.sync.dma_start(out=sb_bdh[:], in_=bdhd[:])
        sb_bdl = const.tile([1, HID], BF)
        nc.sync.dma_start(out=sb_bdl[:], in_=bdld[:])

        persist = ctx.enter_context(tc.tile_pool(name="persist", bufs=1))
        kT = [persist.tile([HD, S], BF, name=f"kT{h}") for h in range(NH_LOC)]
        vnat = persist.tile([HD, 16, 4 * HD], BF)   # [p, sb, hl*128+d]
        qT = persist.tile([HD, NH_LOC, 512], BF)    # current quarter only

        # attention pools (open for the whole run)
        expp = ctx.enter_context(tc.tile_pool(name="expp", bufs=4))
        recp = ctx.enter_context(tc.tile_pool(name="recp", bufs=2))
        ctxp = ctx.enter_context(tc.tile_pool(name="ctxp", bufs=2))
        sc_ps = ctx.enter_context(
            tc.tile_pool(name="sc_ps", bufs=2, space="PSUM"))
        ctx_ps = ctx.enter_context(
            tc.tile_pool(name="ctx_ps", bufs=1, space="PSUM"))
        bc_ps = ctx.enter_context(
            tc.tile_pool(name="bc_ps", bufs=1, space="PSUM"))


        def attention(q):
            q0 = 512 * q
            for hl in range(NH_LOC):
                slope = sb_slopes[:, hl:hl + 1]
                sl = _surv(hl, q)
                ps_ctx = ctx_ps.tile([HD, 512], F32, name="ps_ctx")
                ps_bc = bc_ps.tile([HD, 512], F32, name="ps_bc")
                exs = {}

                def flush(i, first, last):
                    ex, skt, vs0, ve = exs.pop(i)
                    nc.tensor.matmul(
                        ps_ctx[:, vs0:ve],
                        vnat[:, skt, hl * HD:(hl + 1) * HD],
                        ex[:, vs0:ve], start=first, stop=last)
                    nc.tensor.matmul(
                        ps_bc[:, vs0:ve], ones128[:],
                        ex[:, vs0:ve], start=first, stop=last)

                for i, (skt, vs0, ve) in enumerate(sl):
                    o = skt - 4 * q
                    ps = sc_ps.tile([HD, 512], F32, name="ps_sc")
                    nc.tensor.matmul(
                        ps[:, vs0:ve],
                        kT[hl][:, skt * HD:(skt + 1) * HD],
                        qT[:, hl, vs0:ve], start=True, stop=True)
                    c0 = vs0 - o * HD + 384
                    nc.vector.scalar_tensor_tensor(
                        ps[:, vs0:ve], tmat[:, c0:c0 + (ve - vs0)], slope,
                        ps[:, vs0:ve], Alu.mult, Alu.add)
                    ex = expp.tile([HD, 512], BF, name="ex")
                    nc.scalar.activation(ex[:, vs0:ve], ps[:, vs0:ve], Act.Exp)
                    exs[i] = (ex, skt, vs0, ve)
                    if i >= 2:
                        flush(i - 2, i - 2 == 0, False)
                n = len(sl)
                for i in (n - 2, n - 1):
                    if i >= 0 and i in exs:
                        flush(i, i == 0, i == n - 1)

                tln = recp.tile([HD, 512], F32, name="tln")
                nc.scalar.activation(tln[:], ps_bc[:], Act.Ln)
                rec = recp.tile([HD, 512], F32, name="rec")
                nc.scalar.activation(rec[:], tln[:], Act.Exp, scale=-1.0)
                csb = ctxp.tile([HD, 512], BF, name="csb")
                nc.vector.tensor_tensor(csb[:], ps_ctx[:], rec[:], Alu.mult)
                for j in (0, 1):
                    nc.sync.dma_start(
                        out=a2a_in[hl // 2][2 * q + j, hl % 2],
                        in_=csb[:, j * SROW:(j + 1) * SROW])
                if q == 3 and hl == 1:
                    nc.gpsimd.collective_compute(
                        "AllToAll", Alu.bypass,
                        replica_groups=[list(range(NCORES))],
                        ins=[a2a_in[0][:]], outs=[a2a_out[0][:]])
            if q == 3:
                nc.gpsimd.collective_compute(
                    "AllToAll", Alu.bypass,
                    replica_groups=[list(range(NCORES))],
                    ins=[a2a_in[1][:]], outs=[a2a_out[1][:]])

        # ---------- phase 1: QKV + attention, interleaved per quarter ----
        with (
            tc.tile_pool(name="hT_pool", bufs=2) as hT_pool,
            tc.tile_pool(name="wqk_pool", bufs=8) as wqk_pool,
            tc.tile_pool(name="wv_pool", bufs=1) as wv_pool,
            tc.tile_pool(name="qkv_ps", bufs=1, space="PSUM") as qkv_ps,
        ):
            for q in range(4):
                ht = hT_pool.tile([HD, KT, 512], BF, name="ht")
                for cg in range(4):
                    nc.sync.dma_start(
                        out=ht[:, cg * 8:(cg + 1) * 8, :],
                        in_=hTd[:, cg * 8:(cg + 1) * 8, 512 * q:512 * q + 512])
                wv = wv_pool.tile([HD, KT, 4 * HD], BF, name="wv")
                for cg in range(4):
                    nc.scalar.dma_start(
                        out=wv[:, cg * 8:(cg + 1) * 8, :],
                        in_=wvd[:, cg * 8:(cg + 1) * 8, :])

                # q/k sweeps: heads (0,1) then (2,3)
                for grp in range(2):
                    psl = [qkv_ps.tile([HD, 512], F32, name=f"qk{i}", bufs=1)
                           for i in range(4)]
                    for kt in range(KT):
                        wq = wqk_pool.tile([HD, 4 * HD], BF, name="wq")
                        nc.sync.dma_start(
                            out=wq[:],
                            in_=wqkd[:, kt, grp * 512:(grp + 1) * 512])
                        for i in range(4):
                            nc.tensor.matmul(
                                psl[i][:], wq[:, i * HD:(i + 1) * HD],
                                ht[:, kt, :],
                                start=(kt == 0), stop=(kt == KT - 1))
                    for i in range(4):
                        hl = grp * 2 + i // 2
                        isq = i % 2 == 0
                        f = hl * 2 + (0 if isq else 1)
                        if isq:
                            dest = qT[:, hl, :]
                        else:
                            dest = kT[hl][:, 512 * q:512 * q + 512]
                        nc.scalar.activation(
                            dest, psl[i][:], Act.Identity,
                            bias=sb_bqk[:, f:f + 1])

                # V sweep: natural layout, hT blocks stationary
                for sb in range(4):
                    psv = sc_ps.tile([HD, 512], F32, name="ps_sc")
                    for kt in range(KT):
                        nc.tensor.matmul(
                            psv[:], ht[:, kt, sb * HD:(sb + 1) * HD],
                            wv[:, kt, :], start=(kt == 0), stop=(kt == KT - 1))
                    nc.scalar.copy(vnat[:, 4 * q + sb, :], psv[:])

                if q < 3:
                    attention(q)

        # ---------- phase 2: last attention quarter + dense ----------
        with (
            tc.tile_pool(name="wd_pool", bufs=2) as wd_pool,
            tc.tile_pool(name="dns_sb", bufs=1) as dns_sb,
            tc.tile_pool(name="osb_pool", bufs=4) as osb_pool,
            tc.tile_pool(name="dns_ps", bufs=2, space="PSUM") as dns_ps,
        ):
            # prefetch first dense weight chunk, then overlap attention(3)
            wd0 = wd_pool.tile([HD, KT, 512], BF, name="wd")
            nc.scalar.dma_start(out=wd0[:], in_=wdd[0])
            attention(3)
            crecv = dns_sb.tile([HD, KT, SROW], BF)
            for i in range(NCORES):
                for p in range(2):
                    nc.scalar.dma_start(
                        out=crecv[:, i * NH_LOC + p * 2:
                                  i * NH_LOC + p * 2 + 2, :],
                        in_=a2a_out[p][i].rearrange("l p s -> p l s"))
            p0 = [ft for ft in range(KT) if ft % 4 in (0, 1)]
            p1 = [ft for ft in range(KT) if ft % 4 in (2, 3)]
            for oc in range(8):
                wdc = wd0 if oc == 0 else wd_pool.tile(
                    [HD, KT, 512], BF, name="wd")
                if oc > 0:
                    nc.scalar.dma_start(out=wdc[:], in_=wdd[oc])
                psd = [dns_ps.tile([HD, 512], F32, name=f"psd{st}")
                       for st in range(2)]
                passes = [p0, p1] if oc == 0 else [p0 + p1]
                first = True
                for ftset in passes:
                    for st in range(2):
                        for ft in ftset:
                            nc.tensor.matmul(
                                psd[st][:],
                                crecv[:, ft, st * HD:(st + 1) * HD],
                                wdc[:, ft, :], start=first and ft == ftset[0],
                                stop=False)
                    first = False
                for st in range(2):
                    o0 = oc * 512
                    nc.tensor.matmul(psd[st][:], ones1[:],
                                     sb_bdh[:, o0:o0 + 512],
                                     start=False, stop=False)
                    nc.tensor.matmul(psd[st][:], ones1[:],
                                     sb_bdl[:, o0:o0 + 512],
                                     start=False, stop=True)
                    osb = osb_pool.tile([HD, 512], F32, name="osb")
                    nc.scalar.copy(osb[:], psd[st][:])
                    nc.sync.dma_start(
                        out=out[st * HD:(st + 1) * HD, o0:o0 + 512],
                        in_=osb[:])
    nc.compile()
    return nc


def _prep_shards(hidden_states, alibi, w_qkv, b_qkv, w_dense, b_dense):
    bf16 = ml_dtypes.bfloat16
    hidden = np.asarray(hidden_states, dtype=np.float32).reshape(S, HID)
    hT = np.ascontiguousarray(hidden.T).astype(bf16)       # [HID, S]
    hTd = np.ascontiguousarray(hT.reshape(KT, HD, S).transpose(1, 0, 2))
    al = np.asarray(alibi, dtype=np.float32).reshape(NH, S)
    w = np.asarray(w_qkv, dtype=np.float32)                # [3H, H]
    b = np.asarray(b_qkv, dtype=np.float32)
    wd = np.asarray(w_dense, dtype=np.float32)             # [H, H]
    bd = np.asarray(b_dense, dtype=np.float32)

    wT = np.ascontiguousarray(w.T)                         # [H, 3H]

    # fold v-bias into dense bias: out = wd @ (ctx + bv) + bd
    bv_full = np.zeros(HID, np.float32)
    for g in range(NH):
        bv_full[g * HD:(g + 1) * HD] = b[g * 3 * HD + 2 * HD:
                                         g * 3 * HD + 3 * HD]
    bdf = bd + wd @ bv_full
    bdh = bdf.astype(bf16)
    bdl = (bdf - bdh.astype(np.float32)).astype(bf16)

    # wide masked distance table  T[a, c'] = a-c if a<=c else NEG, c=c'-384
    a = np.arange(HD)[:, None]
    cp = np.arange(TW)[None, :] - 384
    tmat = np.where(a <= cp, (a - cp).astype(np.float32), np.float32(NEG))
    tmat = tmat.astype(ml_dtypes.float16 if False else np.float16)

    in_maps = []
    for c in range(NCORES):
        heads = [c + 8 * hl for hl in range(NH_LOC)]
        # q/k weights, feature-major [p, kt, (hl, qk, d)]
        wqk = np.empty((KT, HD, 8 * HD), np.float32)
        wv = np.empty((KT, HD, 4 * HD), np.float32)
        bqk = np.empty((HD, 8), np.float32)
        for hl, g in enumerate(heads):
            r = g * 3 * HD
            wqk[:, :, hl * 2 * HD:hl * 2 * HD + HD] = \
                (wT[:, r:r + HD] * INV_NORM).reshape(KT, HD, HD)
            wqk[:, :, hl * 2 * HD + HD:(hl + 1) * 2 * HD] = \
                wT[:, r + HD:r + 2 * HD].reshape(KT, HD, HD)
            wv[:, :, hl * HD:(hl + 1) * HD] = \
                wT[:, r + 2 * HD:r + 3 * HD].reshape(KT, HD, HD)
            bqk[:, hl * 2] = b[r:r + HD] * INV_NORM
            bqk[:, hl * 2 + 1] = b[r + HD:r + 2 * HD]
        slopes = np.repeat(al[heads, 1:2].T, HD, axis=0)   # [128, 4]

        # dense weights: rows by global head of ft = i*4 + 2p + j,
        # g(ft) = 8*(ft%4) + ft//4 ; o-chunks of 512
        wdT = wd.T                                         # [f, o]
        wdr = np.empty((8, HD, KT, 512), np.float32)
        for ft in range(KT):
            g = 8 * (ft % 4) + ft // 4
            blk = wdT[g * HD:(g + 1) * HD]                 # [128, 4096]
            wdr[:, :, ft, :] = blk.reshape(HD, 8, 512).transpose(1, 0, 2)

        in_maps.append({
            "hT": hTd,
            "wqk": np.ascontiguousarray(
                wqk.transpose(1, 0, 2)).astype(bf16),
            "wv": np.ascontiguousarray(wv.transpose(1, 0, 2)).astype(bf16),
            "bqk": np.ascontiguousarray(bqk),
            "tmat": tmat,
            "slopes": np.ascontiguousarray(slopes.astype(np.float32)),
            "wd": np.ascontiguousarray(wdr).astype(bf16),
            "bdh": bdh.reshape(1, HID),
            "bdl": bdl.reshape(1, HID),
        })
    return in_maps


def _unshard(res):
    # out rows of core c are s in [c*256, (c+1)*256)
    outp = np.concatenate([res.results[c]["out"] for c in range(NCORES)],
                          axis=0)
    return outp.reshape(B, S, HID).astype(np.float32)


def kernel(hidden_states, alibi, w_qkv, b_qkv, w_dense, b_dense):
    _ensure_axon_hooks()
    from concourse import bass_utils

    if "nc" not in _CACHE:
        _CACHE["nc"] = _build_nc()
    nc = _CACHE["nc"]
    in_maps = _prep_shards(hidden_states, alibi, w_qkv, b_qkv,
                           w_dense, b_dense)
    trace = bool(os.environ.get("BLOOM_TRACE"))
    res = bass_utils.run_bass_kernel_spmd(
        nc, in_maps, core_ids=list(range(NCORES)), trace=trace)
    kernel._last_results = res
    kernel._last_exec_ns = res.exec_time_ns
    return _unshard(res)


# revision 20
# speedup vs baseline: 1.3231x; 1.0289x over previous
"""BloomAttention (B=1, S=2048, HID=4096, NH=32) on 8 Trainium2 NeuronCores.

v2 strategy (tensor-parallel over heads):
  - Heads assigned per core by octile slots: core c owns global heads
    {c, c+8, c+16, c+24}. Slot k's ALiBi slope is at most 2^-(2k+2), so
    attention blocks farther than D_slot = 30/slope_min from the causal
    diagonal contribute < ~1e-7 relative mass and are skipped (structure is
    baked uniformly across cores; slope VALUES stay per-core runtime data).
  - hidden is transposed on HOST (hT [HID, S]) - zero on-device transposes.
  - Flash order: per 512-quarter, QKV matmuls then attention. q/k produced
    feature-major (weights stationary); V produced sequence-major directly
    (hT blocks stationary, w_v moving) so PV needs no transpose.
  - Scores in transposed layout [sk, sq]: alibi+causal+per-query shift via a
    single wide masked distance table T[a,c] = (a-c <= 0 ? a-c : -60000),
    applied by one DVE scalar_tensor_tensor per (column-striped) block; exp
    on ACT; denominator via ones[128,128] stationary matmul accumulating a
    broadcast row-sum in PSUM; 1/d computed as exp(-ln d) on ACT (one act
    table set serves exp+ln+identity); V-bias folded into b_dense on host.
  - AllToAll (2x 1MB) swaps head-shards for sequence-shards; first fires
    after local heads {c, c+8} finish. Dense: w_dense streamed in o-chunks,
    crecv stationary (LDW amortized over o), p0/p1 split on the first chunk
    to hide the second collective; bias added via hi/lo bf16 ones-matmuls.
"""

import math
import os
import sys
import types
from contextlib import ExitStack

import numpy as np
import ml_dtypes

B, S, HID, NH, HD = 1, 2048, 4096, 32, 128
NCORES = 8
NH_LOC = NH // NCORES            # 4 heads per core (slots)
SROW = S // NCORES               # 256 output rows per core
INV_NORM = 1.0 / math.sqrt(HD)
KT = HID // HD                   # 32 k tiles
TW = 2432                        # wide distance-table columns
NEG = -60000.0
DSLOT = [120, 480, 1920, 2048]   # per-slot causal stripe depth (30/slope_min)

_CACHE = {}


def _ensure_axon_hooks():
    try:
        import antenv  # noqa: F401

        extra = "/opt/trn_rl_repo/antenv"
        if os.path.isdir(extra) and extra not in antenv.__path__:
            antenv.__path__.append(extra)
        import antenv.axon_hooks  # noqa: F401
    except Exception:
        hook = None
        try:
            from trn_agent_boot.trn_boot import _ntff_profile_via_ctypes

            hook = _ntff_profile_via_ctypes("/opt/axon/libaxon_pjrt.so")
        except Exception:
            hook = None
        m = types.ModuleType("antenv.axon_hooks")
        m._hook = hook
        m.get_axon_ntff_profile_hook = lambda: m._hook
        m.set_axon_ntff_profile_hook = lambda h: setattr(m, "_hook", h)
        sys.modules["antenv.axon_hooks"] = m


def _surv(hl, q):
    """Surviving (skt, vs0, ve) column stripes for local head hl, quarter q.

    First surviving block is widened to the full 512 columns so its
    start=True matmul initialises every PSUM column of ps_ctx / ps_bc.
    """
    D = DSLOT[hl]
    sq0 = 512 * q
    out = []
    for skt in range(4 * q + 4):
        vs0 = max(0, 128 * skt - sq0)
        ve = min(512, 128 * skt + 128 + D - sq0)
        if ve <= vs0:
            continue
        out.append([skt, vs0, ve])
    out[0][1] = 0
    out[0][2] = 512
    return out


def _build_nc():
    import concourse.bass as bass  # noqa: F401
    import concourse.mybir as mybir
    from concourse import bacc, tile

    BF = mybir.dt.bfloat16
    F16 = mybir.dt.float16
    F32 = mybir.dt.float32
    Alu = mybir.AluOpType
    Act = mybir.ActivationFunctionType

    nc = bacc.Bacc(None, target_bir_lowering=False, num_devices=NCORES)
    with tile.TileContext(nc) as tc, ExitStack() as ctx:
        dram = ctx.enter_context(tc.tile_pool(name="dram", bufs=1, space="DRAM"))

        def din(name, shape, dt):
            return dram.tile(shape, dt, kind="ExternalInput", name=name,
                             uniquify=False)

        hTd = din("hT", [HD, KT, S], BF)
        wqkd = din("wqk", [HD, KT, 8 * HD], BF)
        wvd = din("wv", [HD, KT, 4 * HD], BF)
        bqkd = din("bqk", [HD, 8], F32)
        tmatd = din("tmat", [HD, TW], F16)
        slopesd = din("slopes", [HD, NH_LOC], F32)
        wdd = din("wd", [8, HD, 16, 1024], BF)
        bdhd = din("bdh", [1, HID], BF)
        bdld = din("bdl", [1, HID], BF)
        out = dram.tile([SROW, HID], F32, kind="ExternalOutput", name="out",
                        uniquify=False)
        a2a_in = [dram.tile([NCORES, HD, SROW], BF, name=f"a2a_in{p}")
                  for p in range(NH_LOC)]
        a2a_out = [dram.tile([NCORES, HD, SROW], BF, name=f"a2a_out{p}")
                   for p in range(NH_LOC)]

        # ---------- persistent SBUF ----------
        const = ctx.enter_context(tc.tile_pool(name="const", bufs=1))
        sb_bqk = const.tile([HD, 8], F32)
        nc.scalar.dma_start(out=sb_bqk[:], in_=bqkd[:])
        sb_slopes = const.tile([HD, NH_LOC], F32)
        nc.scalar.dma_start(out=sb_slopes[:], in_=slopesd[:])
        tmat = const.tile([HD, TW], F16)
        nc.scalar.dma_start(out=tmat[:], in_=tmatd[:])
        ones128 = const.tile([HD, HD], BF)
        nc.vector.memset(ones128[:], 1.0)
        ones1 = const.tile([1, HD], BF)
        nc.vector.memset(ones1[:], 1.0)
        sb_bdh = const.tile([1, HID], BF)
        nc.scalar.dma_start(out=sb_bdh[:], in_=bdhd[:])
        sb_bdl = const.tile([1, HID], BF)
        nc.scalar.dma_start(out=sb_bdl[:], in_=bdld[:])

        persist = ctx.enter_context(tc.tile_pool(name="persist", bufs=1))
        kT = [persist.tile([HD, S], BF, name=f"kT{h}") for h in range(NH_LOC)]
        vnat = persist.tile([HD, 16, 4 * HD], BF)   # [p, sb, hl*128+d]
        qT = persist.tile([HD, NH_LOC, 512], BF)    # current quarter only

        # attention pools (open for the whole run)
        expp = ctx.enter_context(tc.tile_pool(name="expp", bufs=4))
        recp = ctx.enter_context(tc.tile_pool(name="recp", bufs=2))
        ctxp = ctx.enter_context(tc.tile_pool(name="ctxp", bufs=2))
        sc_ps = ctx.enter_context(
            tc.tile_pool(name="sc_ps", bufs=2, space="PSUM"))
        ctx_ps = ctx.enter_context(
            tc.tile_pool(name="ctx_ps", bufs=1, space="PSUM"))
        bc_ps = ctx.enter_context(
            tc.tile_pool(name="bc_ps", bufs=1, space="PSUM"))


        def attention(q):
            q0 = 512 * q
            for hl in range(NH_LOC):
                slope = sb_slopes[:, hl:hl + 1]
                sl = _surv(hl, q)
                ps_ctx = ctx_ps.tile([HD, 512], F32, name="ps_ctx")
                ps_bc = bc_ps.tile([HD, 512], F32, name="ps_bc")
                exs = {}

                def flush(i, first, last):
                    ex, skt, vs0, ve = exs.pop(i)
                    nc.tensor.matmul(
                        ps_ctx[:, vs0:ve],
                        vnat[:, skt, hl * HD:(hl + 1) * HD],
                        ex[:, vs0:ve], start=first, stop=last)
                    nc.tensor.matmul(
                        ps_bc[:, vs0:ve], ones128[:],
                        ex[:, vs0:ve], start=first, stop=last)

                for i, (skt, vs0, ve) in enumerate(sl):
                    o = skt - 4 * q
                    ps = sc_ps.tile([HD, 512], F32, name="ps_sc")
                    nc.tensor.matmul(
                        ps[:, vs0:ve],
                        kT[hl][:, skt * HD:(skt + 1) * HD],
                        qT[:, hl, vs0:ve], start=True, stop=True)
                    c0 = vs0 - o * HD + 384
                    nc.vector.scalar_tensor_tensor(
                        ps[:, vs0:ve], tmat[:, c0:c0 + (ve - vs0)], slope,
                        ps[:, vs0:ve], Alu.mult, Alu.add)
                    ex = expp.tile([HD, 512], BF, name="ex")
                    nc.scalar.activation(ex[:, vs0:ve], ps[:, vs0:ve], Act.Exp)
                    exs[i] = (ex, skt, vs0, ve)
                    if i >= 2:
                        flush(i - 2, i - 2 == 0, False)
                n = len(sl)
                for i in (n - 2, n - 1):
                    if i >= 0 and i in exs:
                        flush(i, i == 0, i == n - 1)

                rec = recp.tile([HD, 512], F32, name="rec")
                nc.vector.reciprocal_approx_fast(rec[:], ps_bc[:])
                csb = ctxp.tile([HD, 512], BF, name="csb")
                nc.vector.tensor_tensor(csb[:], ps_ctx[:], rec[:], Alu.mult)
                for j in (0, 1):
                    nc.sync.dma_start(
                        out=a2a_in[hl][2 * q + j],
                        in_=csb[:, j * SROW:(j + 1) * SROW])
                if q == 3:
                    nc.gpsimd.collective_compute(
                        "AllToAll", Alu.bypass,
                        replica_groups=[list(range(NCORES))],
                        ins=[a2a_in[hl][:]], outs=[a2a_out[hl][:]])

        # ---------- phase 1: QKV + attention, interleaved per quarter ----
        with (
            tc.tile_pool(name="hT_pool", bufs=2) as hT_pool,
            tc.tile_pool(name="wqk_pool", bufs=8) as wqk_pool,
            tc.tile_pool(name="wv_pool", bufs=1) as wv_pool,
            tc.tile_pool(name="qkv_ps", bufs=1, space="PSUM") as qkv_ps,
        ):
            for q in range(4):
                ht = hT_pool.tile([HD, KT, 512], BF, name="ht")
                for cg in range(4):
                    nc.sync.dma_start(
                        out=ht[:, cg * 8:(cg + 1) * 8, :],
                        in_=hTd[:, cg * 8:(cg + 1) * 8, 512 * q:512 * q + 512])
                wv = wv_pool.tile([HD, KT, 4 * HD], BF, name="wv")
                for cg in range(4):
                    nc.scalar.dma_start(
                        out=wv[:, cg * 8:(cg + 1) * 8, :],
                        in_=wvd[:, cg * 8:(cg + 1) * 8, :])

                # q/k sweeps: heads (0,1) then (2,3)
                for grp in range(2):
                    psl = [qkv_ps.tile([HD, 512], F32, name=f"qk{i}", bufs=1)
                           for i in range(4)]
                    for kt in range(KT):
                        if kt % 4 == 0:
                            wq = wqk_pool.tile([HD, 4, 4 * HD], BF, name="wq")
                            nc.sync.dma_start(
                                out=wq[:],
                                in_=wqkd[:, kt:kt + 4,
                                         grp * 512:(grp + 1) * 512])
                        for i in range(4):
                            nc.tensor.matmul(
                                psl[i][:],
                                wq[:, kt % 4, i * HD:(i + 1) * HD],
                                ht[:, kt, :],
                                start=(kt == 0), stop=(kt == KT - 1))
                    for i in range(4):
                        hl = grp * 2 + i // 2
                        isq = i % 2 == 0
                        f = hl * 2 + (0 if isq else 1)
                        if isq:
                            dest = qT[:, hl, :]
                        else:
                            dest = kT[hl][:, 512 * q:512 * q + 512]
                        nc.scalar.activation(
                            dest, psl[i][:], Act.Identity,
                            bias=sb_bqk[:, f:f + 1])

                # V sweep: natural layout, hT blocks stationary
                for sb in range(4):
                    psv = sc_ps.tile([HD, 512], F32, name="ps_sc")
                    for kt in range(KT):
                        nc.tensor.matmul(
                            psv[:], ht[:, kt, sb * HD:(sb + 1) * HD],
                            wv[:, kt, :], start=(kt == 0), stop=(kt == KT - 1))
                    nc.scalar.copy(vnat[:, 4 * q + sb, :], psv[:])

                if q < 3:
                    attention(q)

        # ---------- phase 2: last attention quarter + dense ----------
        with (
            tc.tile_pool(name="wd_pool", bufs=2) as wd_pool,
            tc.tile_pool(name="dns_sb", bufs=1) as dns_sb,
            tc.tile_pool(name="osb_pool", bufs=4) as osb_pool,
            tc.tile_pool(name="dns_ps", bufs=2, space="PSUM") as dns_ps,
        ):
            # prefetch first dense weight chunk, then overlap attention(3)
            wd0 = wd_pool.tile([HD, 16, 1024], BF, name="wd")
            nc.scalar.dma_start(out=wd0[:], in_=wdd[0])
            attention(3)
            crecv = dns_sb.tile([HD, KT, SROW], BF)
            for hl in range(NH_LOC):
                nc.scalar.dma_start(
                    out=crecv[:, hl:KT:NH_LOC, :],
                    in_=a2a_out[hl].rearrange("i p s -> p i s"))
            for oc in range(4):
                psd = [[dns_ps.tile([HD, 512], F32, name=f"psd{st}{oh}",
                                    bufs=1) for oh in range(2)]
                       for st in range(2)]
                first = True
                for ftc in range(2):
                    f0 = ftc * 16
                    wdc = wd0 if oc == 0 and ftc == 0 else wd_pool.tile(
                        [HD, 16, 1024], BF, name="wd")
                    if oc > 0 or ftc > 0:
                        nc.scalar.dma_start(out=wdc[:], in_=wdd[oc * 2 + ftc])
                    if oc == 0 and ftc == 0:
                        passes = [list(range(m, 16, NH_LOC))
                                  for m in range(NH_LOC)]
                    else:
                        passes = [list(range(f0, f0 + 16))]
                    for ftset in passes:
                        for st in range(2):
                            for ft in ftset:
                                for oh in range(2):
                                    nc.tensor.matmul(
                                        psd[st][oh][:],
                                        crecv[:, ft, st * HD:(st + 1) * HD],
                                        wdc[:, ft - f0,
                                            oh * 512:(oh + 1) * 512],
                                        start=first and ft == ftset[0],
                                        stop=False)
                        first = False
                for st in range(2):
                    for oh in range(2):
                        o0 = oc * 1024 + oh * 512
                        nc.tensor.matmul(psd[st][oh][:], ones1[:],
                                         sb_bdh[:, o0:o0 + 512],
                                         start=False, stop=False)
                        nc.tensor.matmul(psd[st][oh][:], ones1[:],
                                         sb_bdl[:, o0:o0 + 512],
                                         start=False, stop=True)
                        osb = osb_pool.tile([HD, 512], F32, name="osb")
                        nc.scalar.copy(osb[:], psd[st][oh][:])
                        nc.sync.dma_start(
                            out=out[st * HD:(st + 1) * HD, o0:o0 + 512],
                            in_=osb[:])
    nc.compile()
    return nc


def _prep_shards(hidden_states, alibi, w_qkv, b_qkv, w_dense, b_dense):
    bf16 = ml_dtypes.bfloat16
    hidden = np.asarray(hidden_states, dtype=np.float32).reshape(S, HID)
    hT = np.ascontiguousarray(hidden.T).astype(bf16)       # [HID, S]
    hTd = np.ascontiguousarray(hT.reshape(KT, HD, S).transpose(1, 0, 2))
    al = np.asarray(alibi, dtype=np.float32).reshape(NH, S)
    w = np.asarray(w_qkv, dtype=np.float32)                # [3H, H]
    b = np.asarray(b_qkv, dtype=np.float32)
    wd = np.asarray(w_dense, dtype=np.float32)             # [H, H]
    bd = np.asarray(b_dense, dtype=np.float32)

    wT = np.ascontiguousarray(w.T)                         # [H, 3H]

    # fold v-bias into dense bias: out = wd @ (ctx + bv) + bd
    bv_full = np.zeros(HID, np.float32)
    for g in range(NH):
        bv_full[g * HD:(g + 1) * HD] = b[g * 3 * HD + 2 * HD:
                                         g * 3 * HD + 3 * HD]
    bdf = bd + wd @ bv_full
    bdh = bdf.astype(bf16)
    bdl = (bdf - bdh.astype(np.float32)).astype(bf16)

    # wide masked distance table  T[a, c'] = a-c if a<=c else NEG, c=c'-384
    a = np.arange(HD)[:, None]
    cp = np.arange(TW)[None, :] - 384
    tmat = np.where(a <= cp, (a - cp).astype(np.float32), np.float32(NEG))
    tmat = tmat.astype(ml_dtypes.float16 if False else np.float16)

    in_maps = []
    for c in range(NCORES):
        heads = [c + 8 * hl for hl in range(NH_LOC)]
        # q/k weights, feature-major [p, kt, (hl, qk, d)]
        wqk = np.empty((KT, HD, 8 * HD), np.float32)
        wv = np.empty((KT, HD, 4 * HD), np.float32)
        bqk = np.empty((HD, 8), np.float32)
        for hl, g in enumerate(heads):
            r = g * 3 * HD
            wqk[:, :, hl * 2 * HD:hl * 2 * HD + HD] = \
                (wT[:, r:r + HD] * INV_NORM).reshape(KT, HD, HD)
            wqk[:, :, hl * 2 * HD + HD:(hl + 1) * 2 * HD] = \
                wT[:, r + HD:r + 2 * HD].reshape(KT, HD, HD)
            wv[:, :, hl * HD:(hl + 1) * HD] = \
                wT[:, r + 2 * HD:r + 3 * HD].reshape(KT, HD, HD)
            bqk[:, hl * 2] = b[r:r + HD] * INV_NORM
            bqk[:, hl * 2 + 1] = b[r + HD:r + 2 * HD]
        slopes = np.repeat(al[heads, 1:2].T, HD, axis=0)   # [128, 4]

        # dense weights: rows by global head of ft = i*4 + 2p + j,
        # g(ft) = 8*(ft%4) + ft//4 ; o-chunks of 512
        wdT = wd.T                                         # [f, o]
        wdr4 = np.empty((4, HD, KT, 1024), np.float32)
        for ft in range(KT):
            g = 8 * (ft % 4) + ft // 4
            blk = wdT[g * HD:(g + 1) * HD]                 # [128, 4096]
            wdr4[:, :, ft, :] = blk.reshape(HD, 4, 1024).transpose(1, 0, 2)
        wdr = wdr4.reshape(4, HD, 2, 16, 1024).transpose(
            0, 2, 1, 3, 4).reshape(8, HD, 16, 1024)

        in_maps.append({
            "hT": hTd,
            "wqk": np.ascontiguousarray(
                wqk.transpose(1, 0, 2)).astype(bf16),
            "wv": np.ascontiguousarray(wv.transpose(1, 0, 2)).astype(bf16),
            "bqk": np.ascontiguousarray(bqk),
            "tmat": tmat,
            "slopes": np.ascontiguousarray(slopes.astype(np.float32)),
            "wd": np.ascontiguousarray(wdr).astype(bf16),
            "bdh": bdh.reshape(1, HID),
            "bdl": bdl.reshape(1, HID),
        })
    return in_maps


def _unshard(res):
    # out rows of core c are s in [c*256, (c+1)*256)
    outp = np.concatenate([res.results[c]["out"] for c in range(NCORES)],
                          axis=0)
    return outp.reshape(B, S, HID).astype(np.float32)


def kernel(hidden_states, alibi, w_qkv, b_qkv, w_dense, b_dense):
    _ensure_axon_hooks()
    from concourse import bass_utils

    if "nc" not in _CACHE:
        _CACHE["nc"] = _build_nc()
    nc = _CACHE["nc"]
    in_maps = _prep_shards(hidden_states, alibi, w_qkv, b_qkv,
                           w_dense, b_dense)
    trace = bool(os.environ.get("BLOOM_TRACE"))
    res = bass_utils.run_bass_kernel_spmd(
        nc, in_maps, core_ids=list(range(NCORES)), trace=trace)
    kernel._last_results = res
    kernel._last_exec_ns = res.exec_time_ns
    return _unshard(res)
